# revision 10
# baseline (speedup 1.0000x reference)
"""Trainium2 Bass kernel for nn_Jointer: per-sample masked cosine-similarity.

out[b] = relu(l2norm(source[b]) @ l2norm(target[b]).T) * (mask_src[b] outer mask_tar[b])

Sharding: data-parallel over batch B=8 -> one sample per NeuronCore.

Strategy (memory-bound problem; rel-err budget 2e-2 permits bf16 I/O):
- Host casts source/target to bf16 (halves input DMA bytes); kernel writes a
  bf16 output that the host upcasts to f32 (halves the dominant 16 MB output
  stream). Norm statistics and matmul accumulation stay fp32; measured end-to-
  end rel err ~3.3e-3.
- Engine roles: GpSimd does the SBUF-side elementwise prep (squares, reduces,
  scale*mask) so the DVE/ACT FIFOs stay clear for the PSUM-drain relu ops;
  DVE does reciprocals + half the relu ops + half the transpose copies; ACT
  does sqrt + the other halves. PE does transposes + matmuls only.
- t operands are scaled by rsqrt(ss)*mask BEFORE the PE transpose; s operands
  are transposed RAW and their rsqrt(ss)*mask scale is fused into the
  PSUM->SBUF relu pass (per-row scalar), so the s-transposes depend only on
  the s input DMA.
- Inputs load via two queues in parallel (sync HWDGE + gpsimd SWDGE); outputs
  stream on the sync queue as a column-major "band" over rows 0-3 (each band
  segment needs only the t-blocks transposed so far), then rows 4-15 go
  row-major with 1 MB row-pair DMAs.
- PSUM->SBUF relu ops work on [128,1024] two-bank tiles to amortize the ACT
  engine's fixed overhead; ops alternate ACT/DVE. Transpose PSUM tiles are
  padded to a full bank so PE writes never share a bank with ACT/DVE reads.
"""

import numpy as np
import ml_dtypes

import concourse.bass as bass
from concourse import bacc
import concourse.mybir as mybir
import concourse.tile as tile
from concourse.bass_utils import run_bass_kernel_spmd
from concourse.masks import make_identity

F32 = mybir.dt.float32
BF16 = mybir.dt.bfloat16
AF = mybir.ActivationFunctionType
ALU = mybir.AluOpType
AX = mybir.AxisListType

S = 2048  # source tokens per sample
T = 2048  # target tokens per sample
D = 128  # feature dim (= contraction dim = partitions)
P = 128  # partitions
SB = S // P  # 16 source token blocks
TB = T // P  # 16 target token blocks

BF = ml_dtypes.bfloat16


def build_nc() -> bass.Bass:
    nc = bacc.Bacc(trn_type="TRN2")

    src = nc.dram_tensor("src", [S, D], BF16, kind="ExternalInput")
    tgt = nc.dram_tensor("tgt", [T, D], BF16, kind="ExternalInput")
    # maskf[p, k]: k in [0,16) source-block masks, k in [16,32) target-block
    # masks; value for token 128*k + p.
    maskf = nc.dram_tensor("maskf", [P, SB + TB], F32, kind="ExternalInput")
    out = nc.dram_tensor("out", [S, T], BF16, kind="ExternalOutput")

    src_r = src.rearrange("(k p) d -> p k d", p=P)
    tgt_r = tgt.rearrange("(k p) d -> p k d", p=P)
    out_pm = out.rearrange("(m p) n -> p m n", p=P)  # [P, 16, 2048]

    with tile.TileContext(nc) as tc:
        with (
            tc.tile_pool(name="singles", bufs=1) as singles,
            tc.tile_pool(name="inbuf", bufs=1) as inbuf,
            tc.tile_pool(name="sq", bufs=2) as sqp,
            tc.tile_pool(name="scl", bufs=4) as sclp,
            tc.tile_pool(name="psmm", bufs=2, space="PSUM") as psmm,
            tc.tile_pool(name="bandp", bufs=1) as bandp,
            tc.tile_pool(name="outp", bufs=3) as outp,
        ):
            ident = singles.tile([P, P], BF16)
            make_identity(nc, ident)

            # First ACT-stream instruction must be a Sqrt so the compiler
            # loads the sqrt table set (which also contains relu/copy) once;
            # otherwise a Copy-first stream loads a different set and the
            # switch lands on the ramp critical path.
            sqrt_warm = singles.tile([P, 1], F32)
            nc.scalar.activation(out=sqrt_warm, in_=ident[:, 0:1], func=AF.Sqrt)

            # PE warmup: HAM up-clocks only after ~4us of dense matmul
            # activity, and the window is free-running. Burn dummy matmuls in
            # the preamble/input-DMA shadow so the real matmuls start at the
            # warm clock. (Transposes don't count toward HAM activity.)
            warm_mv = singles.tile([P, 512], BF16)
            nc.gpsimd.memset(warm_mv, 0.0)

            def pe_warm(n):
                for _ in range(n):
                    pw = psmm.tile([P, 2048], F32, tag="mm", name="warm")
                    nc.tensor.matmul(
                        pw[:, 0:512], ident, warm_mv, start=True, stop=True
                    )

            mask_sb = singles.tile([P, SB + TB], F32)

            s_nat = inbuf.tile([P, SB, D], BF16)
            t_nat = inbuf.tile([P, TB, D], BF16)
            sT = inbuf.tile([P, S], BF16)  # [D, s tokens] raw (scale in relu)
            tT = inbuf.tile([P, T], BF16)  # [D, t tokens] normalized+masked

            ss_t = singles.tile([P, TB], F32)
            rc_t = singles.tile([P, TB], F32)
            rq_t = singles.tile([P, TB], F32)
            ss_s = singles.tile([P, SB], F32)
            rc_s = singles.tile([P, SB], F32)
            rq_s = singles.tile([P, SB], F32)
            s_scl = singles.tile([P, SB], F32)  # rsqrt * mask, per s block
            rqm_t = singles.tile([P, TB], F32)  # rsqrt * mask, per t block

            # ---- input DMAs: two queues in parallel. sync HWDGE carries the
            # t stream + s03 (FIFO order == drain order); gpsimd SWDGE carries
            # the mask + s tail concurrently.
            nc.sync.dma_start(out=t_nat[:, 0:4, :], in_=tgt_r[:, 0:4, :])
            nc.sync.dma_start(out=s_nat[:, 0:4, :], in_=src_r[:, 0:4, :])
            nc.sync.dma_start(out=t_nat[:, 4:8, :], in_=tgt_r[:, 4:8, :])
            nc.sync.dma_start(out=t_nat[:, 8:16, :], in_=tgt_r[:, 8:16, :])
            nc.gpsimd.dma_start(out=mask_sb, in_=maskf.rearrange("p k -> p k"))
            nc.gpsimd.dma_start(out=s_nat[:, 4:16, :], in_=src_r[:, 4:16, :])

            def norm(x_nat, ss, rc, rq, lo, n, tag):
                """sum-of-squares (gpsimd) -> 1/x (DVE) -> sqrt (ACT)."""
                sq = sqp.tile([P, n, D], BF16, tag="sq", name=f"sq_{tag}{lo}")
                nc.vector.tensor_mul(
                    out=sq, in0=x_nat[:, lo : lo + n, :], in1=x_nat[:, lo : lo + n, :]
                )
                nc.vector.reduce_sum(out=ss[:, lo : lo + n], in_=sq, axis=AX.X)
                nc.vector.reciprocal(out=rc[:, lo : lo + n], in_=ss[:, lo : lo + n])
                nc.scalar.activation(
                    out=rq[:, lo : lo + n], in_=rc[:, lo : lo + n], func=AF.Sqrt
                )

            def xpose_t(lo, n, scl_eng=None):
                """masked-rsqrt scale of a whole t group in ONE broadcast
                multiply (stride-0 free-dim AP), then PE-transpose + copy."""
                nc.vector.tensor_mul(
                    out=rqm_t[:, lo : lo + n],
                    in0=rq_t[:, lo : lo + n],
                    in1=mask_sb[:, SB + lo : SB + lo + n],
                )
                xs = sclp.tile([P, n, D], BF16, tag="scl", name=f"xs{lo}")
                rqb = (
                    rqm_t[:, lo : lo + n]
                    .rearrange("p (n o) -> p n o", o=1)
                    .broadcast_to([P, n, D])
                )
                (scl_eng or nc.vector).tensor_mul(
                    out=xs, in0=t_nat[:, lo : lo + n, :], in1=rqb
                )
                ps = psmm.tile([P, 1024], BF16, tag="mm", name=f"xpt{lo}")
                for j in range(n):
                    nc.tensor.transpose(ps[:, j * P : (j + 1) * P], xs[:, j, :], ident)
                half = n * P // 2
                base = lo * P
                nc.scalar.copy(out=tT[:, base : base + half], in_=ps[:, 0:half])
                nc.vector.tensor_copy(
                    out=tT[:, base + half : base + n * P], in_=ps[:, half : n * P]
                )

            def xpose_s(lo, n):
                """PE-transpose raw s blocks (depends only on the s DMA)."""
                ps = psmm.tile([P, 1024], BF16, tag="mm", name=f"xps{lo}")
                for j in range(n):
                    k = lo + j
                    nc.tensor.transpose(ps[:, j * P : (j + 1) * P], s_nat[:, k, :], ident)
                half = n * P // 2
                base = lo * P
                nc.scalar.copy(out=sT[:, base : base + half], in_=ps[:, 0:half])
                nc.vector.tensor_copy(
                    out=sT[:, base + half : base + n * P], in_=ps[:, half : n * P]
                )

            def s_mask(lo, n):
                nc.vector.tensor_mul(
                    out=s_scl[:, lo : lo + n],
                    in0=rq_s[:, lo : lo + n],
                    in1=mask_sb[:, lo : lo + n],
                )

            alt = [0]

            def out_op(dst, ps_ap, m):
                """relu(scale * psum) -> bf16 SBUF, alternating ACT/DVE."""
                if alt[0] % 2 == 0:
                    nc.scalar.activation(
                        out=dst, in_=ps_ap, func=AF.Relu, scale=s_scl[:, m : m + 1]
                    )
                else:
                    nc.vector.tensor_scalar(
                        out=dst,
                        in0=ps_ap,
                        scalar1=s_scl[:, m : m + 1],
                        scalar2=0.0,
                        op0=ALU.mult,
                        op1=ALU.max,
                    )
                alt[0] += 1

            # band output tiles for rows 0-3: [P, m-pair, T]
            ob01 = bandp.tile([P, 2, T], BF16, name="ob01")
            ob23 = bandp.tile([P, 2, T], BF16, name="ob23")
            band_obs = [(ob01, 0), (ob23, 2)]

            def band_seg(c0):
                """rows 0-3, columns [c0, c0+512)."""
                for ob, mlo in band_obs:
                    ps = psmm.tile([P, 2048], F32, tag="mm", name=f"b{c0}_{mlo}")
                    for i in range(2):
                        m = mlo + i
                        nc.tensor.matmul(
                            ps[:, i * 512 : (i + 1) * 512],
                            sT[:, m * P : (m + 1) * P],
                            tT[:, c0 : c0 + 512],
                            start=True,
                            stop=True,
                        )
                    for i in range(2):
                        m = mlo + i
                        out_op(
                            ob[:, i, c0 : c0 + 512], ps[:, i * 512 : (i + 1) * 512], m
                        )
                    nc.sync.dma_start(
                        out=out_pm[:, mlo : mlo + 2, c0 : c0 + 512],
                        in_=ob[:, :, c0 : c0 + 512],
                    )

            def band_segwide():
                """rows 0-3, columns [1024, 2048)."""
                for ob, mlo in band_obs:
                    for i in range(2):
                        m = mlo + i
                        ps = psmm.tile([P, 2048], F32, tag="mm", name=f"bD_{m}")
                        nc.tensor.matmul(
                            ps[:, 0:512],
                            sT[:, m * P : (m + 1) * P],
                            tT[:, 1024:1536],
                            start=True,
                            stop=True,
                        )
                        nc.tensor.matmul(
                            ps[:, 512:1024],
                            sT[:, m * P : (m + 1) * P],
                            tT[:, 1536:2048],
                            start=True,
                            stop=True,
                        )
                        out_op(ob[:, i, 1024:2048], ps[:, 0:1024], m)
                    nc.sync.dma_start(
                        out=out_pm[:, mlo : mlo + 2, 1024:2048],
                        in_=ob[:, :, 1024:2048],
                    )

            def row_single(m):
                obs = outp.tile([P, T], BF16, tag="obs", name=f"obs{m}")
                ps = psmm.tile([P, 2048], F32, tag="mm", name=f"r{m}")
                for h in range(4):
                    c = h * 512
                    nc.tensor.matmul(
                        ps[:, c : c + 512],
                        sT[:, m * P : (m + 1) * P],
                        tT[:, c : c + 512],
                        start=True,
                        stop=True,
                    )
                out_op(obs, ps, m)
                nc.sync.dma_start(out=out_pm[:, m, :], in_=obs)

            def row_pair(mlo):
                """rows mlo, mlo+1 row-major; one 1 MB pair DMA."""
                obp = outp.tile([P, 2, T], BF16, tag="obp", name=f"obp{mlo}")
                for i in range(2):
                    m = mlo + i
                    ps = psmm.tile([P, 2048], F32, tag="mm", name=f"r{m}")
                    for h in range(4):
                        c = h * 512
                        nc.tensor.matmul(
                            ps[:, c : c + 512],
                            sT[:, m * P : (m + 1) * P],
                            tT[:, c : c + 512],
                            start=True,
                            stop=True,
                        )
                    out_op(obp[:, i, :], ps, m)
                nc.sync.dma_start(out=out_pm[:, mlo : mlo + 2, :], in_=obp)

            # ---- emission order == per-engine FIFO order. Per engine, ops
            # are emitted in the order their inputs become ready so no engine
            # head-of-line-blocks on a semaphore while later work is ready.
            pe_warm(8)
            norm(t_nat, ss_t, rc_t, rq_t, 0, 4, "t")  # t0-3
            xpose_s(0, 4)  # PE: only needs the s03 DMA
            pe_warm(2)
            xpose_t(0, 4)
            norm(s_nat, ss_s, rc_s, rq_s, 0, 4, "s")  # s0-3 (for s_scl)
            s_mask(0, 4)
            pe_warm(2)
            norm(t_nat, ss_t, rc_t, rq_t, 4, 4, "t")  # t4-7
            band_seg(0)
            xpose_t(4, 4, scl_eng=nc.gpsimd)
            norm(t_nat, ss_t, rc_t, rq_t, 8, 4, "t")
            band_seg(512)
            xpose_t(8, 4, scl_eng=nc.gpsimd)
            norm(t_nat, ss_t, rc_t, rq_t, 12, 4, "t")
            xpose_t(12, 4)
            norm(s_nat, ss_s, rc_s, rq_s, 4, 4, "s")
            s_mask(4, 4)
            band_segwide()
            xpose_s(4, 4)
            row_pair(4)
            norm(s_nat, ss_s, rc_s, rq_s, 8, 4, "s")
            s_mask(8, 4)
            row_pair(6)
            xpose_s(8, 4)
            row_pair(8)
            norm(s_nat, ss_s, rc_s, rq_s, 12, 4, "s")
            s_mask(12, 4)
            row_pair(10)
            xpose_s(12, 4)
            row_pair(12)
            row_single(14)
            row_single(15)

    nc.compile()
    return nc


_NC_CACHE = None


def _get_nc():
    global _NC_CACHE
    if _NC_CACHE is None:
        _NC_CACHE = build_nc()
    return _NC_CACHE


def kernel(source, target, mask_src, mask_tar, **run_kwargs):
    source = np.asarray(source, dtype=np.float32)
    target = np.asarray(target, dtype=np.float32)
    mask_src = np.asarray(mask_src)
    mask_tar = np.asarray(mask_tar)
    B = source.shape[0]

    in_maps = []
    for b in range(B):
        msf = mask_src[b].astype(np.float32).reshape(SB, P).T
        mtf = mask_tar[b].astype(np.float32).reshape(TB, P).T
        mk = np.ascontiguousarray(np.concatenate([msf, mtf], axis=1))
        in_maps.append(
            {
                "src": np.ascontiguousarray(source[b].astype(BF)),
                "tgt": np.ascontiguousarray(target[b].astype(BF)),
                "maskf": mk,
            }
        )

    nc = _get_nc()
    res = run_bass_kernel_spmd(nc, in_maps, core_ids=list(range(B)), **run_kwargs)
    out = np.stack(
        [np.asarray(r["out"]).astype(np.float32) for r in res.results], axis=0
    )
    if run_kwargs.get("trace"):
        kernel.last_results = res
    return out


# revision 11
# speedup vs baseline: 1.0965x; 1.0965x over previous
"""Trainium2 Bass kernel for nn_Jointer: per-sample masked cosine-similarity.

out[b] = relu(l2norm(source[b]) @ l2norm(target[b]).T) * (mask_src[b] outer mask_tar[b])

Sharding: data-parallel over batch B=8 -> one sample per NeuronCore.

Strategy (memory-bound problem; rel-err budget 2e-2 permits bf16 I/O):
- Host casts source/target to bf16 (halves input DMA bytes); kernel writes a
  bf16 output that the host upcasts to f32 (halves the dominant 16 MB output
  stream). Norm statistics and matmul accumulation stay fp32; measured end-to-
  end rel err ~3.3e-3.
- Engine roles: GpSimd does the SBUF-side elementwise prep (squares, reduces,
  scale*mask) so the DVE/ACT FIFOs stay clear for the PSUM-drain relu ops;
  DVE does reciprocals + half the relu ops + half the transpose copies; ACT
  does sqrt + the other halves. PE does transposes + matmuls only.
- t operands are scaled by rsqrt(ss)*mask BEFORE the PE transpose; s operands
  are transposed RAW and their rsqrt(ss)*mask scale is fused into the
  PSUM->SBUF relu pass (per-row scalar), so the s-transposes depend only on
  the s input DMA.
- Inputs load via two queues in parallel (sync HWDGE + gpsimd SWDGE); outputs
  stream on the sync queue as a column-major "band" over rows 0-3 (each band
  segment needs only the t-blocks transposed so far), then rows 4-15 go
  row-major with 1 MB row-pair DMAs.
- PSUM->SBUF relu ops work on [128,1024] two-bank tiles to amortize the ACT
  engine's fixed overhead; ops alternate ACT/DVE. Transpose PSUM tiles are
  padded to a full bank so PE writes never share a bank with ACT/DVE reads.
"""

import numpy as np
import ml_dtypes

import concourse.bass as bass
from concourse import bacc
import concourse.mybir as mybir
import concourse.tile as tile
from concourse.bass_utils import run_bass_kernel_spmd
from concourse.masks import make_identity

F32 = mybir.dt.float32
BF16 = mybir.dt.bfloat16
AF = mybir.ActivationFunctionType
ALU = mybir.AluOpType
AX = mybir.AxisListType

S = 2048  # source tokens per sample
T = 2048  # target tokens per sample
D = 128  # feature dim (= contraction dim = partitions)
P = 128  # partitions
SB = S // P  # 16 source token blocks
TB = T // P  # 16 target token blocks

BF = ml_dtypes.bfloat16


def build_nc() -> bass.Bass:
    nc = bacc.Bacc(trn_type="TRN2")

    src = nc.dram_tensor("src", [S, D], BF16, kind="ExternalInput")
    tgt = nc.dram_tensor("tgt", [T, D], BF16, kind="ExternalInput")
    # maskf[p, k]: k in [0,16) source-block masks, k in [16,32) target-block
    # masks; value for token 128*k + p.
    maskf = nc.dram_tensor("maskf", [P, SB + TB], F32, kind="ExternalInput")
    out = nc.dram_tensor("out", [S, T], BF16, kind="ExternalOutput")

    src_r = src.rearrange("(k p) d -> p k d", p=P)
    tgt_r = tgt.rearrange("(k p) d -> p k d", p=P)
    out_pm = out.rearrange("(m p) n -> p m n", p=P)  # [P, 16, 2048]

    with tile.TileContext(nc) as tc:
        with (
            tc.tile_pool(name="singles", bufs=1) as singles,
            tc.tile_pool(name="inbuf", bufs=1) as inbuf,
            tc.tile_pool(name="sq", bufs=2) as sqp,
            tc.tile_pool(name="scl", bufs=4) as sclp,
            tc.tile_pool(name="pst", bufs=2, space="PSUM") as pst,
            tc.tile_pool(name="psmm", bufs=3, space="PSUM") as psmm,
            tc.tile_pool(name="bandp", bufs=1) as bandp,
            tc.tile_pool(name="outp", bufs=3) as outp,
        ):
            ident = singles.tile([P, P], BF16)
            make_identity(nc, ident)

            # First ACT-stream instruction must be a Sqrt so the compiler
            # loads the sqrt table set (which also contains relu/copy) once;
            # otherwise a Copy-first stream loads a different set and the
            # switch lands on the ramp critical path.
            sqrt_warm = singles.tile([P, 1], F32)
            nc.scalar.activation(out=sqrt_warm, in_=ident[:, 0:1], func=AF.Sqrt)

            # PE warmup: HAM up-clocks only after ~4us of dense matmul
            # activity, and the window is free-running. Burn dummy matmuls in
            # the preamble/input-DMA shadow so the real matmuls start at the
            # warm clock. (Transposes don't count toward HAM activity.)
            warm_mv = singles.tile([P, 512], BF16)
            nc.gpsimd.memset(warm_mv, 0.0)

            def pe_warm(n):
                for _ in range(n):
                    pw = psmm.tile([P, 1024], F32, tag="mm", name="warm")
                    nc.tensor.matmul(
                        pw[:, 0:512], ident, warm_mv, start=True, stop=True
                    )

            mask_sb = singles.tile([P, SB + TB], F32)

            s_nat = inbuf.tile([P, SB, D], BF16)
            t_nat = inbuf.tile([P, TB, D], BF16)
            sT = inbuf.tile([P, S], BF16)  # [D, s tokens] raw (scale in relu)
            tT = inbuf.tile([P, T], BF16)  # [D, t tokens] normalized+masked

            ss_t = singles.tile([P, TB], F32)
            rc_t = singles.tile([P, TB], F32)
            rq_t = singles.tile([P, TB], F32)
            ss_s = singles.tile([P, SB], F32)
            rc_s = singles.tile([P, SB], F32)
            rq_s = singles.tile([P, SB], F32)
            s_scl = singles.tile([P, SB], F32)  # rsqrt * mask, per s block
            rqm_t = singles.tile([P, TB], F32)  # rsqrt * mask, per t block

            # ---- input DMAs: two queues in parallel. sync HWDGE carries the
            # t stream + s03 (FIFO order == drain order); gpsimd SWDGE carries
            # the mask + s tail concurrently.
            nc.sync.dma_start(out=t_nat[:, 0:4, :], in_=tgt_r[:, 0:4, :])
            nc.sync.dma_start(out=s_nat[:, 0:4, :], in_=src_r[:, 0:4, :])
            nc.sync.dma_start(out=t_nat[:, 4:8, :], in_=tgt_r[:, 4:8, :])
            nc.sync.dma_start(out=t_nat[:, 8:16, :], in_=tgt_r[:, 8:16, :])
            nc.gpsimd.dma_start(out=mask_sb, in_=maskf.rearrange("p k -> p k"))
            nc.gpsimd.dma_start(out=s_nat[:, 4:16, :], in_=src_r[:, 4:16, :])

            def norm(x_nat, ss, rc, rq, lo, n, tag):
                """sum-of-squares (gpsimd) -> 1/x (DVE) -> sqrt (ACT)."""
                sq = sqp.tile([P, n, D], BF16, tag="sq", name=f"sq_{tag}{lo}")
                nc.vector.tensor_mul(
                    out=sq, in0=x_nat[:, lo : lo + n, :], in1=x_nat[:, lo : lo + n, :]
                )
                nc.vector.reduce_sum(out=ss[:, lo : lo + n], in_=sq, axis=AX.X)
                nc.vector.reciprocal(out=rc[:, lo : lo + n], in_=ss[:, lo : lo + n])
                nc.scalar.activation(
                    out=rq[:, lo : lo + n], in_=rc[:, lo : lo + n], func=AF.Sqrt
                )

            def xpose_t(lo, n, scl_eng=None):
                """masked-rsqrt scale of a whole t group in ONE broadcast
                multiply (stride-0 free-dim AP), then PE-transpose + copy."""
                nc.vector.tensor_mul(
                    out=rqm_t[:, lo : lo + n],
                    in0=rq_t[:, lo : lo + n],
                    in1=mask_sb[:, SB + lo : SB + lo + n],
                )
                xs = sclp.tile([P, n, D], BF16, tag="scl", name=f"xs{lo}")
                rqb = (
                    rqm_t[:, lo : lo + n]
                    .rearrange("p (n o) -> p n o", o=1)
                    .broadcast_to([P, n, D])
                )
                (scl_eng or nc.vector).tensor_mul(
                    out=xs, in0=t_nat[:, lo : lo + n, :], in1=rqb
                )
                ps = pst.tile([P, 1024], BF16, tag="pst", name=f"xpt{lo}")
                for j in range(n):
                    nc.tensor.transpose(ps[:, j * P : (j + 1) * P], xs[:, j, :], ident)
                half = n * P // 2
                base = lo * P
                nc.scalar.copy(out=tT[:, base : base + half], in_=ps[:, 0:half])
                nc.vector.tensor_copy(
                    out=tT[:, base + half : base + n * P], in_=ps[:, half : n * P]
                )

            def xpose_s(lo, n):
                """PE-transpose raw s blocks (depends only on the s DMA)."""
                ps = pst.tile([P, 1024], BF16, tag="pst", name=f"xps{lo}")
                for j in range(n):
                    k = lo + j
                    nc.tensor.transpose(ps[:, j * P : (j + 1) * P], s_nat[:, k, :], ident)
                half = n * P // 2
                base = lo * P
                nc.scalar.copy(out=sT[:, base : base + half], in_=ps[:, 0:half])
                nc.vector.tensor_copy(
                    out=sT[:, base + half : base + n * P], in_=ps[:, half : n * P]
                )

            def s_mask(lo, n):
                nc.vector.tensor_mul(
                    out=s_scl[:, lo : lo + n],
                    in0=rq_s[:, lo : lo + n],
                    in1=mask_sb[:, lo : lo + n],
                )

            alt = [0]

            def out_op(dst, ps_ap, m):
                """relu(scale * psum) -> bf16 SBUF, alternating ACT/DVE."""
                if alt[0] % 2 == 0:
                    nc.scalar.activation(
                        out=dst, in_=ps_ap, func=AF.Relu, scale=s_scl[:, m : m + 1]
                    )
                else:
                    nc.vector.tensor_scalar(
                        out=dst,
                        in0=ps_ap,
                        scalar1=s_scl[:, m : m + 1],
                        scalar2=0.0,
                        op0=ALU.mult,
                        op1=ALU.max,
                    )
                alt[0] += 1

            # band output tiles for rows 0-3: [P, m-pair, T]
            ob01 = bandp.tile([P, 2, T], BF16, name="ob01")
            ob23 = bandp.tile([P, 2, T], BF16, name="ob23")
            band_obs = [(ob01, 0), (ob23, 2)]

            def band_seg(c0):
                """rows 0-3, columns [c0, c0+512)."""
                for ob, mlo in band_obs:
                    ps = psmm.tile([P, 1024], F32, tag="mm", name=f"b{c0}_{mlo}")
                    for i in range(2):
                        m = mlo + i
                        nc.tensor.matmul(
                            ps[:, i * 512 : (i + 1) * 512],
                            sT[:, m * P : (m + 1) * P],
                            tT[:, c0 : c0 + 512],
                            start=True,
                            stop=True,
                        )
                    for i in range(2):
                        m = mlo + i
                        out_op(
                            ob[:, i, c0 : c0 + 512], ps[:, i * 512 : (i + 1) * 512], m
                        )
                    nc.sync.dma_start(
                        out=out_pm[:, mlo : mlo + 2, c0 : c0 + 512],
                        in_=ob[:, :, c0 : c0 + 512],
                    )

            def band_segwide():
                """rows 0-3, columns [1024, 2048)."""
                for ob, mlo in band_obs:
                    for i in range(2):
                        m = mlo + i
                        ps = psmm.tile([P, 1024], F32, tag="mm", name=f"bD_{m}")
                        nc.tensor.matmul(
                            ps[:, 0:512],
                            sT[:, m * P : (m + 1) * P],
                            tT[:, 1024:1536],
                            start=True,
                            stop=True,
                        )
                        nc.tensor.matmul(
                            ps[:, 512:1024],
                            sT[:, m * P : (m + 1) * P],
                            tT[:, 1536:2048],
                            start=True,
                            stop=True,
                        )
                        out_op(ob[:, i, 1024:2048], ps, m)
                    nc.sync.dma_start(
                        out=out_pm[:, mlo : mlo + 2, 1024:2048],
                        in_=ob[:, :, 1024:2048],
                    )

            def row_single(m):
                obs = outp.tile([P, T], BF16, tag="obs", name=f"obs{m}")
                for h in range(2):
                    c = h * 1024
                    ps = psmm.tile([P, 1024], F32, tag="mm", name=f"r{m}_{h}")
                    nc.tensor.matmul(
                        ps[:, 0:512],
                        sT[:, m * P : (m + 1) * P],
                        tT[:, c : c + 512],
                        start=True,
                        stop=True,
                    )
                    nc.tensor.matmul(
                        ps[:, 512:1024],
                        sT[:, m * P : (m + 1) * P],
                        tT[:, c + 512 : c + 1024],
                        start=True,
                        stop=True,
                    )
                    out_op(obs[:, c : c + 1024], ps, m)
                    nc.sync.dma_start(
                        out=out_pm[:, m, c : c + 1024], in_=obs[:, c : c + 1024]
                    )

            def row_pair(mlo):
                """rows mlo, mlo+1 row-major; one 1 MB pair DMA."""
                obp = outp.tile([P, 2, T], BF16, tag="obp", name=f"obp{mlo}")
                for i in range(2):
                    m = mlo + i
                    for h in range(2):
                        c = h * 1024
                        ps = psmm.tile([P, 1024], F32, tag="mm", name=f"r{m}_{h}")
                        nc.tensor.matmul(
                            ps[:, 0:512],
                            sT[:, m * P : (m + 1) * P],
                            tT[:, c : c + 512],
                            start=True,
                            stop=True,
                        )
                        nc.tensor.matmul(
                            ps[:, 512:1024],
                            sT[:, m * P : (m + 1) * P],
                            tT[:, c + 512 : c + 1024],
                            start=True,
                            stop=True,
                        )
                        out_op(obp[:, i, c : c + 1024], ps, m)
                nc.sync.dma_start(out=out_pm[:, mlo : mlo + 2, :], in_=obp)

            # ---- emission order == per-engine FIFO order. Per engine, ops
            # are emitted in the order their inputs become ready so no engine
            # head-of-line-blocks on a semaphore while later work is ready.
            pe_warm(8)
            norm(t_nat, ss_t, rc_t, rq_t, 0, 4, "t")  # t0-3
            xpose_s(0, 4)  # PE: only needs the s03 DMA
            pe_warm(2)
            xpose_t(0, 4)
            norm(s_nat, ss_s, rc_s, rq_s, 0, 4, "s")  # s0-3 (for s_scl)
            s_mask(0, 4)
            pe_warm(2)
            norm(t_nat, ss_t, rc_t, rq_t, 4, 4, "t")  # t4-7
            band_seg(0)
            xpose_t(4, 4, scl_eng=nc.gpsimd)
            norm(t_nat, ss_t, rc_t, rq_t, 8, 4, "t")
            band_seg(512)
            xpose_t(8, 4, scl_eng=nc.gpsimd)
            norm(t_nat, ss_t, rc_t, rq_t, 12, 4, "t")
            xpose_t(12, 4)
            norm(s_nat, ss_s, rc_s, rq_s, 4, 4, "s")
            s_mask(4, 4)
            band_segwide()
            xpose_s(4, 4)
            row_pair(4)
            norm(s_nat, ss_s, rc_s, rq_s, 8, 4, "s")
            s_mask(8, 4)
            row_pair(6)
            xpose_s(8, 4)
            row_pair(8)
            norm(s_nat, ss_s, rc_s, rq_s, 12, 4, "s")
            s_mask(12, 4)
            row_pair(10)
            xpose_s(12, 4)
            row_pair(12)
            row_single(14)
            row_single(15)

    nc.compile()
    return nc


_NC_CACHE = None


def _get_nc():
    global _NC_CACHE
    if _NC_CACHE is None:
        _NC_CACHE = build_nc()
    return _NC_CACHE


def kernel(source, target, mask_src, mask_tar, **run_kwargs):
    source = np.asarray(source, dtype=np.float32)
    target = np.asarray(target, dtype=np.float32)
    mask_src = np.asarray(mask_src)
    mask_tar = np.asarray(mask_tar)
    B = source.shape[0]

    in_maps = []
    for b in range(B):
        msf = mask_src[b].astype(np.float32).reshape(SB, P).T
        mtf = mask_tar[b].astype(np.float32).reshape(TB, P).T
        mk = np.ascontiguousarray(np.concatenate([msf, mtf], axis=1))
        in_maps.append(
            {
                "src": np.ascontiguousarray(source[b].astype(BF)),
                "tgt": np.ascontiguousarray(target[b].astype(BF)),
                "maskf": mk,
            }
        )

    nc = _get_nc()
    res = run_bass_kernel_spmd(nc, in_maps, core_ids=list(range(B)), **run_kwargs)
    out = np.stack(
        [np.asarray(r["out"]).astype(np.float32) for r in res.results], axis=0
    )
    if run_kwargs.get("trace"):
        kernel.last_results = res
    return out


# revision 12
# speedup vs baseline: 1.1189x; 1.0204x over previous
"""Trainium2 Bass kernel for nn_Jointer: per-sample masked cosine-similarity.

out[b] = relu(l2norm(source[b]) @ l2norm(target[b]).T) * (mask_src[b] outer mask_tar[b])

Sharding: data-parallel over batch B=8 -> one sample per NeuronCore.

Strategy (memory-bound problem; rel-err budget 2e-2 permits bf16 I/O):
- Host casts source/target to bf16 (halves input DMA bytes); kernel writes a
  bf16 output that the host upcasts to f32 (halves the dominant 16 MB output
  stream). Norm statistics and matmul accumulation stay fp32; measured end-to-
  end rel err ~3.3e-3.
- Engine roles: GpSimd does the SBUF-side elementwise prep (squares, reduces,
  scale*mask) so the DVE/ACT FIFOs stay clear for the PSUM-drain relu ops;
  DVE does reciprocals + half the relu ops + half the transpose copies; ACT
  does sqrt + the other halves. PE does transposes + matmuls only.
- t operands are scaled by rsqrt(ss)*mask BEFORE the PE transpose; s operands
  are transposed RAW and their rsqrt(ss)*mask scale is fused into the
  PSUM->SBUF relu pass (per-row scalar), so the s-transposes depend only on
  the s input DMA.
- Inputs load via two queues in parallel (sync HWDGE + gpsimd SWDGE); outputs
  stream on the sync queue as a column-major "band" over rows 0-3 (each band
  segment needs only the t-blocks transposed so far), then rows 4-15 go
  row-major with 1 MB row-pair DMAs.
- PSUM->SBUF relu ops work on [128,1024] two-bank tiles to amortize the ACT
  engine's fixed overhead; ops alternate ACT/DVE. Transpose PSUM tiles are
  padded to a full bank so PE writes never share a bank with ACT/DVE reads.
"""

import numpy as np
import ml_dtypes

import concourse.bass as bass
from concourse import bacc
import concourse.mybir as mybir
import concourse.tile as tile
from concourse.bass_utils import run_bass_kernel_spmd
from concourse.masks import make_identity

F32 = mybir.dt.float32
BF16 = mybir.dt.bfloat16
AF = mybir.ActivationFunctionType
ALU = mybir.AluOpType
AX = mybir.AxisListType

S = 2048  # source tokens per sample
T = 2048  # target tokens per sample
D = 128  # feature dim (= contraction dim = partitions)
P = 128  # partitions
SB = S // P  # 16 source token blocks
TB = T // P  # 16 target token blocks

BF = ml_dtypes.bfloat16


def build_nc() -> bass.Bass:
    nc = bacc.Bacc(trn_type="TRN2")

    src = nc.dram_tensor("src", [S, D], BF16, kind="ExternalInput")
    tgt = nc.dram_tensor("tgt", [T, D], BF16, kind="ExternalInput")
    # maskf[p, k]: k in [0,16) source-block masks, k in [16,32) target-block
    # masks; value for token 128*k + p.
    maskf = nc.dram_tensor("maskf", [P, SB + TB], F32, kind="ExternalInput")
    out = nc.dram_tensor("out", [S, T], BF16, kind="ExternalOutput")

    src_r = src.rearrange("(k p) d -> p k d", p=P)
    tgt_r = tgt.rearrange("(k p) d -> p k d", p=P)
    out_pm = out.rearrange("(m p) n -> p m n", p=P)  # [P, 16, 2048]

    with tile.TileContext(nc) as tc:
        with (
            tc.tile_pool(name="singles", bufs=1) as singles,
            tc.tile_pool(name="inbuf", bufs=1) as inbuf,
            tc.tile_pool(name="sq", bufs=2) as sqp,
            tc.tile_pool(name="scl", bufs=4) as sclp,
            tc.tile_pool(name="pst", bufs=2, space="PSUM") as pst,
            tc.tile_pool(name="psmm", bufs=3, space="PSUM") as psmm,
            tc.tile_pool(name="bandp", bufs=1) as bandp,
            tc.tile_pool(name="outp", bufs=3) as outp,
        ):
            ident = singles.tile([P, P], BF16)
            make_identity(nc, ident)

            # First ACT-stream instruction must be a Sqrt so the compiler
            # loads the sqrt table set (which also contains relu/copy) once;
            # otherwise a Copy-first stream loads a different set and the
            # switch lands on the ramp critical path.
            sqrt_warm = singles.tile([P, 1], F32)
            nc.scalar.activation(out=sqrt_warm, in_=ident[:, 0:1], func=AF.Sqrt)

            # PE warmup: HAM up-clocks only after ~4us of dense matmul
            # activity, and the window is free-running. Burn dummy matmuls in
            # the preamble/input-DMA shadow so the real matmuls start at the
            # warm clock. (Transposes don't count toward HAM activity.)
            warm_mv = singles.tile([P, 512], BF16)
            nc.gpsimd.memset(warm_mv, 0.0)

            def pe_warm(n):
                for _ in range(n):
                    pw = psmm.tile([P, 1024], F32, tag="mm", name="warm")
                    nc.tensor.matmul(
                        pw[:, 0:512], ident, warm_mv, start=True, stop=True
                    )

            mask_sb = singles.tile([P, SB + TB], F32)

            s_nat = inbuf.tile([P, SB, D], BF16)
            t_nat = inbuf.tile([P, TB, D], BF16)
            sT = inbuf.tile([P, S], BF16)  # [D, s tokens] raw (scale in relu)
            tT = inbuf.tile([P, T], BF16)  # [D, t tokens] normalized+masked

            ss_t = singles.tile([P, TB], F32)
            rc_t = singles.tile([P, TB], F32)
            rq_t = singles.tile([P, TB], F32)
            ss_s = singles.tile([P, SB], F32)
            rc_s = singles.tile([P, SB], F32)
            rq_s = singles.tile([P, SB], F32)
            s_scl = singles.tile([P, SB], F32)  # rsqrt * mask, per s block
            rqm_t = singles.tile([P, TB], F32)  # rsqrt * mask, per t block

            # ---- input DMAs: two queues in parallel. sync HWDGE carries the
            # t stream + s03 (FIFO order == drain order); gpsimd SWDGE carries
            # the mask + s tail concurrently.
            nc.sync.dma_start(out=mask_sb, in_=maskf.rearrange("p k -> p k"))
            nc.sync.dma_start(out=t_nat[:, 0:4, :], in_=tgt_r[:, 0:4, :])
            nc.sync.dma_start(out=s_nat[:, 0:4, :], in_=src_r[:, 0:4, :])
            nc.sync.dma_start(out=t_nat[:, 4:8, :], in_=tgt_r[:, 4:8, :])
            nc.sync.dma_start(out=t_nat[:, 8:16, :], in_=tgt_r[:, 8:16, :])
            nc.sync.dma_start(out=s_nat[:, 4:16, :], in_=src_r[:, 4:16, :])

            def norm(x_nat, ss, rc, rq, lo, n, tag):
                """sum-of-squares (gpsimd) -> 1/x (DVE) -> sqrt (ACT)."""
                sq = sqp.tile([P, n, D], BF16, tag="sq", name=f"sq_{tag}{lo}")
                nc.vector.tensor_mul(
                    out=sq, in0=x_nat[:, lo : lo + n, :], in1=x_nat[:, lo : lo + n, :]
                )
                nc.vector.reduce_sum(out=ss[:, lo : lo + n], in_=sq, axis=AX.X)
                nc.vector.reciprocal(out=rc[:, lo : lo + n], in_=ss[:, lo : lo + n])
                nc.scalar.activation(
                    out=rq[:, lo : lo + n], in_=rc[:, lo : lo + n], func=AF.Sqrt
                )

            def xpose_t(lo, n, scl_eng=None):
                """masked-rsqrt scale of a whole t group in ONE broadcast
                multiply (stride-0 free-dim AP), then PE-transpose + copy."""
                nc.vector.tensor_mul(
                    out=rqm_t[:, lo : lo + n],
                    in0=rq_t[:, lo : lo + n],
                    in1=mask_sb[:, SB + lo : SB + lo + n],
                )
                xs = sclp.tile([P, n, D], BF16, tag="scl", name=f"xs{lo}")
                rqb = (
                    rqm_t[:, lo : lo + n]
                    .rearrange("p (n o) -> p n o", o=1)
                    .broadcast_to([P, n, D])
                )
                (scl_eng or nc.gpsimd).tensor_mul(
                    out=xs, in0=t_nat[:, lo : lo + n, :], in1=rqb
                )
                ps = pst.tile([P, 1024], BF16, tag="pst", name=f"xpt{lo}")
                for j in range(n):
                    nc.tensor.transpose(ps[:, j * P : (j + 1) * P], xs[:, j, :], ident)
                base = lo * P
                nc.scalar.copy(out=tT[:, base : base + n * P], in_=ps[:, 0 : n * P])

            def xpose_s(lo, n):
                """PE-transpose raw s blocks (depends only on the s DMA)."""
                ps = pst.tile([P, 1024], BF16, tag="pst", name=f"xps{lo}")
                for j in range(n):
                    k = lo + j
                    nc.tensor.transpose(ps[:, j * P : (j + 1) * P], s_nat[:, k, :], ident)
                base = lo * P
                nc.scalar.copy(out=sT[:, base : base + n * P], in_=ps[:, 0 : n * P])

            def s_mask(lo, n):
                nc.vector.tensor_mul(
                    out=s_scl[:, lo : lo + n],
                    in0=rq_s[:, lo : lo + n],
                    in1=mask_sb[:, lo : lo + n],
                )

            alt = [0]

            def out_op(dst, ps_ap, m):
                """relu(scale * psum) -> bf16 SBUF, alternating ACT/DVE."""
                if alt[0] % 2 == 0:
                    nc.scalar.activation(
                        out=dst, in_=ps_ap, func=AF.Relu, scale=s_scl[:, m : m + 1]
                    )
                else:
                    nc.vector.tensor_scalar(
                        out=dst,
                        in0=ps_ap,
                        scalar1=s_scl[:, m : m + 1],
                        scalar2=0.0,
                        op0=ALU.mult,
                        op1=ALU.max,
                    )
                alt[0] += 1

            # band output tiles for rows 0-3: [P, m-pair, T]
            ob01 = bandp.tile([P, 2, T], BF16, name="ob01")
            ob23 = bandp.tile([P, 2, T], BF16, name="ob23")
            band_obs = [(ob01, 0), (ob23, 2)]

            def band_seg(c0):
                """rows 0-3, columns [c0, c0+512)."""
                for ob, mlo in band_obs:
                    ps = psmm.tile([P, 1024], F32, tag="mm", name=f"b{c0}_{mlo}")
                    for i in range(2):
                        m = mlo + i
                        nc.tensor.matmul(
                            ps[:, i * 512 : (i + 1) * 512],
                            sT[:, m * P : (m + 1) * P],
                            tT[:, c0 : c0 + 512],
                            start=True,
                            stop=True,
                        )
                    for i in range(2):
                        m = mlo + i
                        out_op(
                            ob[:, i, c0 : c0 + 512], ps[:, i * 512 : (i + 1) * 512], m
                        )
                    nc.sync.dma_start(
                        out=out_pm[:, mlo : mlo + 2, c0 : c0 + 512],
                        in_=ob[:, :, c0 : c0 + 512],
                    )

            def band_segwide():
                """rows 0-3, columns [1024, 2048)."""
                for ob, mlo in band_obs:
                    for i in range(2):
                        m = mlo + i
                        ps = psmm.tile([P, 1024], F32, tag="mm", name=f"bD_{m}")
                        nc.tensor.matmul(
                            ps[:, 0:512],
                            sT[:, m * P : (m + 1) * P],
                            tT[:, 1024:1536],
                            start=True,
                            stop=True,
                        )
                        nc.tensor.matmul(
                            ps[:, 512:1024],
                            sT[:, m * P : (m + 1) * P],
                            tT[:, 1536:2048],
                            start=True,
                            stop=True,
                        )
                        out_op(ob[:, i, 1024:2048], ps, m)
                    nc.sync.dma_start(
                        out=out_pm[:, mlo : mlo + 2, 1024:2048],
                        in_=ob[:, :, 1024:2048],
                    )

            def row_single(m):
                obs = outp.tile([P, T], BF16, tag="obs", name=f"obs{m}")
                for h in range(2):
                    c = h * 1024
                    ps = psmm.tile([P, 1024], F32, tag="mm", name=f"r{m}_{h}")
                    nc.tensor.matmul(
                        ps[:, 0:512],
                        sT[:, m * P : (m + 1) * P],
                        tT[:, c : c + 512],
                        start=True,
                        stop=True,
                    )
                    nc.tensor.matmul(
                        ps[:, 512:1024],
                        sT[:, m * P : (m + 1) * P],
                        tT[:, c + 512 : c + 1024],
                        start=True,
                        stop=True,
                    )
                    out_op(obs[:, c : c + 1024], ps, m)
                    nc.sync.dma_start(
                        out=out_pm[:, m, c : c + 1024], in_=obs[:, c : c + 1024]
                    )

            def row_pair(mlo):
                """rows mlo, mlo+1 row-major; one 1 MB pair DMA."""
                obp = outp.tile([P, 2, T], BF16, tag="obp", name=f"obp{mlo}")
                for i in range(2):
                    m = mlo + i
                    for h in range(2):
                        c = h * 1024
                        ps = psmm.tile([P, 1024], F32, tag="mm", name=f"r{m}_{h}")
                        nc.tensor.matmul(
                            ps[:, 0:512],
                            sT[:, m * P : (m + 1) * P],
                            tT[:, c : c + 512],
                            start=True,
                            stop=True,
                        )
                        nc.tensor.matmul(
                            ps[:, 512:1024],
                            sT[:, m * P : (m + 1) * P],
                            tT[:, c + 512 : c + 1024],
                            start=True,
                            stop=True,
                        )
                        out_op(obp[:, i, c : c + 1024], ps, m)
                nc.sync.dma_start(out=out_pm[:, mlo : mlo + 2, :], in_=obp)

            # ---- emission order == per-engine FIFO order. Per engine, ops
            # are emitted in the order their inputs become ready so no engine
            # head-of-line-blocks on a semaphore while later work is ready.
            pe_warm(8)
            norm(t_nat, ss_t, rc_t, rq_t, 0, 4, "t")  # t0-3
            xpose_s(0, 4)  # PE: only needs the s03 DMA
            pe_warm(2)
            xpose_t(0, 4)
            norm(s_nat, ss_s, rc_s, rq_s, 0, 4, "s")  # s0-3 (for s_scl)
            s_mask(0, 4)
            pe_warm(2)
            norm(t_nat, ss_t, rc_t, rq_t, 4, 4, "t")  # t4-7
            band_seg(0)
            xpose_t(4, 4, scl_eng=nc.gpsimd)
            norm(t_nat, ss_t, rc_t, rq_t, 8, 4, "t")
            band_seg(512)
            xpose_t(8, 4, scl_eng=nc.gpsimd)
            norm(t_nat, ss_t, rc_t, rq_t, 12, 4, "t")
            xpose_t(12, 4)
            norm(s_nat, ss_s, rc_s, rq_s, 4, 4, "s")
            s_mask(4, 4)
            band_segwide()
            xpose_s(4, 4)
            row_pair(4)
            norm(s_nat, ss_s, rc_s, rq_s, 8, 4, "s")
            s_mask(8, 4)
            row_pair(6)
            xpose_s(8, 4)
            row_pair(8)
            norm(s_nat, ss_s, rc_s, rq_s, 12, 4, "s")
            s_mask(12, 4)
            row_pair(10)
            xpose_s(12, 4)
            row_pair(12)
            row_single(14)
            row_single(15)

    nc.compile()
    return nc


_NC_CACHE = None


def _get_nc():
    global _NC_CACHE
    if _NC_CACHE is None:
        _NC_CACHE = build_nc()
    return _NC_CACHE


def kernel(source, target, mask_src, mask_tar, **run_kwargs):
    source = np.asarray(source, dtype=np.float32)
    target = np.asarray(target, dtype=np.float32)
    mask_src = np.asarray(mask_src)
    mask_tar = np.asarray(mask_tar)
    B = source.shape[0]

    in_maps = []
    for b in range(B):
        msf = mask_src[b].astype(np.float32).reshape(SB, P).T
        mtf = mask_tar[b].astype(np.float32).reshape(TB, P).T
        mk = np.ascontiguousarray(np.concatenate([msf, mtf], axis=1))
        in_maps.append(
            {
                "src": np.ascontiguousarray(source[b].astype(BF)),
                "tgt": np.ascontiguousarray(target[b].astype(BF)),
                "maskf": mk,
            }
        )

    nc = _get_nc()
    res = run_bass_kernel_spmd(nc, in_maps, core_ids=list(range(B)), **run_kwargs)
    out = np.stack(
        [np.asarray(r["out"]).astype(np.float32) for r in res.results], axis=0
    )
    if run_kwargs.get("trace"):
        kernel.last_results = res
    return out


# revision 15
# speedup vs baseline: 1.1887x; 1.0624x over previous
"""Trainium2 Bass kernel for nn_Jointer: per-sample masked cosine-similarity.

out[b] = relu(l2norm(source[b]) @ l2norm(target[b]).T) * (mask_src[b] outer mask_tar[b])

Sharding: data-parallel over batch B=8 -> one sample per NeuronCore.

Strategy (memory-bound problem; rel-err budget 2e-2 permits bf16 I/O):
- Host casts source/target to bf16 (halves input DMA bytes); kernel writes a
  bf16 output that the host upcasts to f32 (halves the dominant 16 MB output
  stream). Norm statistics and matmul accumulation stay fp32; measured end-to-
  end rel err ~3.3e-3.
- Engine roles: GpSimd does the SBUF-side elementwise prep (squares, reduces,
  scale*mask) so the DVE/ACT FIFOs stay clear for the PSUM-drain relu ops;
  DVE does reciprocals + half the relu ops + half the transpose copies; ACT
  does sqrt + the other halves. PE does transposes + matmuls only.
- t operands are scaled by rsqrt(ss)*mask BEFORE the PE transpose; s operands
  are transposed RAW and their rsqrt(ss)*mask scale is fused into the
  PSUM->SBUF relu pass (per-row scalar), so the s-transposes depend only on
  the s input DMA.
- Inputs load via two queues in parallel (sync HWDGE + gpsimd SWDGE); outputs
  stream on the sync queue as a column-major "band" over rows 0-3 (each band
  segment needs only the t-blocks transposed so far), then rows 4-15 go
  row-major with 1 MB row-pair DMAs.
- PSUM->SBUF relu ops work on [128,1024] two-bank tiles to amortize the ACT
  engine's fixed overhead; ops alternate ACT/DVE. Transpose PSUM tiles are
  padded to a full bank so PE writes never share a bank with ACT/DVE reads.
"""

import numpy as np
import ml_dtypes

import concourse.bass as bass
from concourse import bacc
import concourse.mybir as mybir
import concourse.tile as tile
from concourse.bass_utils import run_bass_kernel_spmd
from concourse.masks import make_identity

F32 = mybir.dt.float32
BF16 = mybir.dt.bfloat16
AF = mybir.ActivationFunctionType
ALU = mybir.AluOpType
AX = mybir.AxisListType

S = 2048  # source tokens per sample
T = 2048  # target tokens per sample
D = 128  # feature dim (= contraction dim = partitions)
P = 128  # partitions
SB = S // P  # 16 source token blocks
TB = T // P  # 16 target token blocks

BF = ml_dtypes.bfloat16


def build_nc() -> bass.Bass:
    nc = bacc.Bacc(trn_type="TRN2")

    src = nc.dram_tensor("src", [S, D], BF16, kind="ExternalInput")
    tgt = nc.dram_tensor("tgt", [T, D], BF16, kind="ExternalInput")
    # maskf[p, k]: k in [0,16) source-block masks, k in [16,32) target-block
    # masks; value for token 128*k + p.
    maskf = nc.dram_tensor("maskf", [P, SB + TB], F32, kind="ExternalInput")
    out = nc.dram_tensor("out", [S, T], BF16, kind="ExternalOutput")

    src_r = src.rearrange("(k p) d -> p k d", p=P)
    tgt_r = tgt.rearrange("(k p) d -> p k d", p=P)
    out_pm = out.rearrange("(m p) n -> p m n", p=P)  # [P, 16, 2048]

    with tile.TileContext(nc) as tc:
        with (
            tc.tile_pool(name="singles", bufs=1) as singles,
            tc.tile_pool(name="inbuf", bufs=1) as inbuf,
            tc.tile_pool(name="sq", bufs=2) as sqp,
            tc.tile_pool(name="scl", bufs=4) as sclp,
            tc.tile_pool(name="pst", bufs=2, space="PSUM") as pst,
            tc.tile_pool(name="psmm", bufs=3, space="PSUM") as psmm,
            tc.tile_pool(name="bandp", bufs=1) as bandp,
            tc.tile_pool(name="outp", bufs=3) as outp,
        ):
            ident = singles.tile([P, P], BF16)
            make_identity(nc, ident)

            # First ACT-stream instruction must be a Sqrt so the compiler
            # loads the sqrt table set (which also contains relu/copy) once;
            # otherwise a Copy-first stream loads a different set and the
            # switch lands on the ramp critical path.
            sqrt_warm = singles.tile([P, 1], F32)
            nc.scalar.activation(out=sqrt_warm, in_=ident[:, 0:1], func=AF.Sqrt)

            # PE warmup: HAM up-clocks only after ~4us of dense matmul
            # activity, and the window is free-running. Burn dummy matmuls in
            # the preamble/input-DMA shadow so the real matmuls start at the
            # warm clock. (Transposes don't count toward HAM activity.)
            warm_mv = singles.tile([P, 512], BF16)
            nc.gpsimd.memset(warm_mv, 0.0)

            def pe_warm(n):
                for _ in range(n):
                    pw = psmm.tile([P, 1024], F32, tag="mm", name="warm")
                    nc.tensor.matmul(
                        pw[:, 0:512], ident, warm_mv, start=True, stop=True
                    )

            mask_sb = singles.tile([P, SB + TB], F32)

            s_nat = inbuf.tile([P, SB, D], BF16)
            t_nat = inbuf.tile([P, TB, D], BF16)
            sT = inbuf.tile([P, S], BF16)  # [D, s tokens] raw (scale in relu)
            tT = inbuf.tile([P, T], BF16)  # [D, t tokens] normalized+masked

            ss_t = singles.tile([P, TB], F32)
            rc_t = singles.tile([P, TB], F32)
            rq_t = singles.tile([P, TB], F32)
            ss_s = singles.tile([P, SB], F32)
            rc_s = singles.tile([P, SB], F32)
            rq_s = singles.tile([P, SB], F32)
            s_scl = singles.tile([P, SB], F32)  # rsqrt * mask, per s block
            rqm_t = singles.tile([P, TB], F32)  # rsqrt * mask, per t block

            # ---- input DMAs: two queues in parallel. sync HWDGE carries the
            # t stream + s03 (FIFO order == drain order); gpsimd SWDGE carries
            # the mask + s tail concurrently.
            nc.sync.dma_start(out=mask_sb, in_=maskf.rearrange("p k -> p k"))
            nc.sync.dma_start(out=t_nat[:, 0:4, :], in_=tgt_r[:, 0:4, :])
            nc.sync.dma_start(out=s_nat[:, 0:4, :], in_=src_r[:, 0:4, :])
            nc.sync.dma_start(out=t_nat[:, 4:8, :], in_=tgt_r[:, 4:8, :])
            nc.sync.dma_start(out=t_nat[:, 8:16, :], in_=tgt_r[:, 8:16, :])
            nc.sync.dma_start(out=s_nat[:, 4:16, :], in_=src_r[:, 4:16, :])

            def norm(x_nat, ss, rc, rq, lo, n, tag):
                """sum-of-squares (gpsimd) -> 1/x (DVE) -> sqrt (ACT)."""
                sq = sqp.tile([P, n, D], BF16, tag="sq", name=f"sq_{tag}{lo}")
                nc.vector.tensor_mul(
                    out=sq, in0=x_nat[:, lo : lo + n, :], in1=x_nat[:, lo : lo + n, :]
                )
                nc.vector.reduce_sum(out=ss[:, lo : lo + n], in_=sq, axis=AX.X)
                nc.vector.reciprocal(out=rc[:, lo : lo + n], in_=ss[:, lo : lo + n])
                nc.scalar.activation(
                    out=rq[:, lo : lo + n], in_=rc[:, lo : lo + n], func=AF.Sqrt
                )

            def xpose_t(lo, n, scl_eng=None):
                """masked-rsqrt scale of a whole t group in ONE broadcast
                multiply (stride-0 free-dim AP), then PE-transpose + copy."""
                nc.vector.tensor_mul(
                    out=rqm_t[:, lo : lo + n],
                    in0=rq_t[:, lo : lo + n],
                    in1=mask_sb[:, SB + lo : SB + lo + n],
                )
                xs = sclp.tile([P, n, D], BF16, tag="scl", name=f"xs{lo}")
                rqb = (
                    rqm_t[:, lo : lo + n]
                    .rearrange("p (n o) -> p n o", o=1)
                    .broadcast_to([P, n, D])
                )
                (scl_eng or nc.gpsimd).tensor_mul(
                    out=xs, in0=t_nat[:, lo : lo + n, :], in1=rqb
                )
                ps = pst.tile([P, 1024], BF16, tag="pst", name=f"xpt{lo}")
                for j in range(n):
                    nc.tensor.transpose(ps[:, j * P : (j + 1) * P], xs[:, j, :], ident)
                base = lo * P
                nc.scalar.copy(out=tT[:, base : base + n * P], in_=ps[:, 0 : n * P])

            def xpose_s(lo, n):
                """PE-transpose raw s blocks (depends only on the s DMA)."""
                ps = pst.tile([P, 1024], BF16, tag="pst", name=f"xps{lo}")
                for j in range(n):
                    k = lo + j
                    nc.tensor.transpose(ps[:, j * P : (j + 1) * P], s_nat[:, k, :], ident)
                base = lo * P
                nc.scalar.copy(out=sT[:, base : base + n * P], in_=ps[:, 0 : n * P])

            def s_mask(lo, n):
                nc.vector.tensor_mul(
                    out=s_scl[:, lo : lo + n],
                    in0=rq_s[:, lo : lo + n],
                    in1=mask_sb[:, lo : lo + n],
                )

            alt = [0]

            def out_op(dst, ps_ap, m):
                """relu(scale * psum) -> bf16 SBUF, alternating ACT/DVE."""
                if alt[0] % 2 == 0:
                    nc.scalar.activation(
                        out=dst, in_=ps_ap, func=AF.Relu, scale=s_scl[:, m : m + 1]
                    )
                else:
                    nc.vector.tensor_scalar(
                        out=dst,
                        in0=ps_ap,
                        scalar1=s_scl[:, m : m + 1],
                        scalar2=0.0,
                        op0=ALU.mult,
                        op1=ALU.max,
                    )
                alt[0] += 1

            # band output tiles for rows 0-3: [P, m-pair, T]
            ob01 = bandp.tile([P, 2, T], BF16, name="ob01")
            ob23 = bandp.tile([P, 2, T], BF16, name="ob23")
            band_obs = [(ob01, 0), (ob23, 2)]

            def band_seg(c0):
                """rows 0-3, columns [c0, c0+512)."""
                for ob, mlo in band_obs:
                    ps = psmm.tile([P, 1024], F32, tag="mm", name=f"b{c0}_{mlo}")
                    for i in range(2):
                        m = mlo + i
                        nc.tensor.matmul(
                            ps[:, i * 512 : (i + 1) * 512],
                            sT[:, m * P : (m + 1) * P],
                            tT[:, c0 : c0 + 512],
                            start=True,
                            stop=True,
                        )
                    for i in range(2):
                        m = mlo + i
                        out_op(
                            ob[:, i, c0 : c0 + 512], ps[:, i * 512 : (i + 1) * 512], m
                        )
                    nc.sync.dma_start(
                        out=out_pm[:, mlo : mlo + 2, c0 : c0 + 512],
                        in_=ob[:, :, c0 : c0 + 512],
                    )

            def band_segwide():
                """rows 0-3, columns [1024, 2048)."""
                for ob, mlo in band_obs:
                    for i in range(2):
                        m = mlo + i
                        ps = psmm.tile([P, 1024], F32, tag="mm", name=f"bD_{m}")
                        nc.tensor.matmul(
                            ps[:, 0:512],
                            sT[:, m * P : (m + 1) * P],
                            tT[:, 1024:1536],
                            start=True,
                            stop=True,
                        )
                        nc.tensor.matmul(
                            ps[:, 512:1024],
                            sT[:, m * P : (m + 1) * P],
                            tT[:, 1536:2048],
                            start=True,
                            stop=True,
                        )
                        out_op(ob[:, i, 1024:2048], ps, m)
                    nc.sync.dma_start(
                        out=out_pm[:, mlo : mlo + 2, 1024:2048],
                        in_=ob[:, :, 1024:2048],
                    )

            def row_single(m):
                obs = outp.tile([P, T], BF16, tag="obs", name=f"obs{m}")
                for h in range(2):
                    c = h * 1024
                    ps = psmm.tile([P, 1024], F32, tag="mm", name=f"r{m}_{h}")
                    nc.tensor.matmul(
                        ps[:, 0:512],
                        sT[:, m * P : (m + 1) * P],
                        tT[:, c : c + 512],
                        start=True,
                        stop=True,
                    )
                    nc.tensor.matmul(
                        ps[:, 512:1024],
                        sT[:, m * P : (m + 1) * P],
                        tT[:, c + 512 : c + 1024],
                        start=True,
                        stop=True,
                    )
                    out_op(obs[:, c : c + 1024], ps, m)
                    nc.sync.dma_start(
                        out=out_pm[:, m, c : c + 1024], in_=obs[:, c : c + 1024]
                    )

            def row_pair(mlo):
                """rows mlo, mlo+1 row-major; one 1 MB pair DMA."""
                obp = outp.tile([P, 2, T], BF16, tag="obp", name=f"obp{mlo}")
                for i in range(2):
                    m = mlo + i
                    for h in range(2):
                        c = h * 1024
                        ps = psmm.tile([P, 1024], F32, tag="mm", name=f"r{m}_{h}")
                        nc.tensor.matmul(
                            ps[:, 0:512],
                            sT[:, m * P : (m + 1) * P],
                            tT[:, c : c + 512],
                            start=True,
                            stop=True,
                        )
                        nc.tensor.matmul(
                            ps[:, 512:1024],
                            sT[:, m * P : (m + 1) * P],
                            tT[:, c + 512 : c + 1024],
                            start=True,
                            stop=True,
                        )
                        out_op(obp[:, i, c : c + 1024], ps, m)
                nc.sync.dma_start(out=out_pm[:, mlo : mlo + 2, :], in_=obp)

            # ---- emission order == per-engine FIFO order. Per engine, ops
            # are emitted in the order their inputs become ready so no engine
            # head-of-line-blocks on a semaphore while later work is ready.
            pe_warm(8)
            norm(t_nat, ss_t, rc_t, rq_t, 0, 4, "t")  # t0-3
            xpose_s(0, 4)  # PE: only needs the s03 DMA
            pe_warm(2)
            xpose_t(0, 4)
            norm(s_nat, ss_s, rc_s, rq_s, 0, 4, "s")  # s0-3 (for s_scl)
            s_mask(0, 4)
            pe_warm(2)
            norm(t_nat, ss_t, rc_t, rq_t, 4, 4, "t")  # t4-7
            band_seg(0)
            xpose_t(4, 4, scl_eng=nc.gpsimd)
            norm(t_nat, ss_t, rc_t, rq_t, 8, 4, "t")
            band_seg(512)
            xpose_t(8, 4, scl_eng=nc.gpsimd)
            norm(t_nat, ss_t, rc_t, rq_t, 12, 4, "t")
            xpose_t(12, 4)
            norm(s_nat, ss_s, rc_s, rq_s, 4, 4, "s")
            s_mask(4, 4)
            band_segwide()
            xpose_s(4, 4)
            row_pair(4)
            norm(s_nat, ss_s, rc_s, rq_s, 8, 4, "s")
            s_mask(8, 4)
            row_pair(6)
            xpose_s(8, 4)
            row_pair(8)
            norm(s_nat, ss_s, rc_s, rq_s, 12, 4, "s")
            s_mask(12, 4)
            row_pair(10)
            xpose_s(12, 4)
            row_pair(12)
            row_single(14)
            row_single(15)

    nc.compile()
    return nc


_NC_CACHE = None


def _get_nc():
    global _NC_CACHE
    if _NC_CACHE is None:
        _NC_CACHE = build_nc()
    return _NC_CACHE


def kernel(source, target, mask_src, mask_tar, **run_kwargs):
    source = np.asarray(source, dtype=np.float32)
    target = np.asarray(target, dtype=np.float32)
    mask_src = np.asarray(mask_src)
    mask_tar = np.asarray(mask_tar)
    B = source.shape[0]

    in_maps = []
    for b in range(B):
        msf = mask_src[b].astype(np.float32).reshape(SB, P).T
        mtf = mask_tar[b].astype(np.float32).reshape(TB, P).T
        mk = np.ascontiguousarray(np.concatenate([msf, mtf], axis=1))
        in_maps.append(
            {
                "src": np.ascontiguousarray(source[b].astype(BF)),
                "tgt": np.ascontiguousarray(target[b].astype(BF)),
                "maskf": mk,
            }
        )

    nc = _get_nc()
    res = run_bass_kernel_spmd(nc, in_maps, core_ids=list(range(B)), **run_kwargs)
    out = np.stack(
        [np.asarray(r["out"]).astype(np.float32) for r in res.results], axis=0
    )
    if run_kwargs.get("trace"):
        kernel.last_results = res
    return out
